# revision 1
# baseline (speedup 1.0000x reference)
"""DLRM-style recommender kernel on 8 TRN2 NeuronCores.

Sharding: batch-parallel (B=16384 -> 2048/core), embedding tables replicated.
Per-core pipeline (16 tiles of 128 samples):
  - indirect-DMA gather of embedding rows -> emb_sb [128b, 26f*64d]
  - dense 2-layer MLP computed transposed -> embD slot 0 [64d, 128b]
  - PE transposes: 13 chunk transposes -> embT [(f,d), b]; 26 per-feature
    transposes -> embD [64d, 27f, 128b]
  - 128 per-sample gram matmuls (K=64, M=27, N=27) -> PSUM; banded
    extraction (27 g-bands) -> interT chunks [(g,f) rows, b]
  - over-arch: z.T chunks (embT + dembT + interT) against resident W1,
    biases folded in via ones-row matmuls; layers 2-4 with PE transposes
    between layers. All matmul operands fed as float32r (full fp32 data,
    1 cyc/row at N>=256 vs 4 for plain fp32).
"""

import sys

sys.path.insert(0, "/opt/trn_rl_repo")
sys.path.insert(0, "/opt/trn_rl_repo/concourse")

import numpy as np

import concourse.bacc as bacc
import concourse.bass as bass
import concourse.mybir as mybir
import concourse.tile as tile
from concourse.bass_utils import run_bass_kernel_spmd
from concourse.masks import make_identity

B, NF, D, V, NDENSE = 16384, 26, 64, 100000, 13
NCORES = 8
BC = B // NCORES          # 2048 per core
P = 128
NT = BC // P              # 16 tiles
F = NF + 1                # 27 features incl. dense
NPAIR = F * F             # 729 full inter entries (diag weighted 0)
K_EMB = NF * D            # 1664
K1 = K_EMB + D + NPAIR    # 2457 contraction dim of layer 1
H1, H2, H3 = 512, 256, 128

F32 = mybir.dt.float32
F32R = mybir.dt.float32r
I32 = mybir.dt.int32

_cache = {}


def _r(ap):
    """Matmul operand dtype hook (plain f32 for now)."""
    return ap


def _copy(nc, i, out, in_):
    """Alternate PSUM->SBUF copies between DVE and ACT."""
    if i % 2 == 0:
        nc.vector.tensor_copy(out, in_)
    else:
        nc.scalar.copy(out, in_)


def build_nc():
    nc = bacc.Bacc("TRN2", target_bir_lowering=False, debug=False,
                   num_devices=NCORES)

    tab = nc.dram_tensor("tab", [NF * V, D], F32, kind="ExternalInput").ap()
    xT = nc.dram_tensor("xT", [NDENSE, BC], F32, kind="ExternalInput").ap()
    idx = nc.dram_tensor("idx", [BC, NF], I32, kind="ExternalInput").ap()
    dw1 = nc.dram_tensor("dw1", [NDENSE, D], F32, kind="ExternalInput").ap()
    db1 = nc.dram_tensor("db1", [D, 1], F32, kind="ExternalInput").ap()
    dw2 = nc.dram_tensor("dw2", [D, D], F32, kind="ExternalInput").ap()
    db2 = nc.dram_tensor("db2", [D, 1], F32, kind="ExternalInput").ap()
    w1 = nc.dram_tensor("w1", [21 * P, H1], F32, kind="ExternalInput").ap()
    b1 = nc.dram_tensor("b1", [1, H1], F32, kind="ExternalInput").ap()
    w2 = nc.dram_tensor("w2", [H1, H2], F32, kind="ExternalInput").ap()
    b2 = nc.dram_tensor("b2", [1, H2], F32, kind="ExternalInput").ap()
    w3 = nc.dram_tensor("w3", [H2, H3], F32, kind="ExternalInput").ap()
    b3 = nc.dram_tensor("b3", [1, H3], F32, kind="ExternalInput").ap()
    w4 = nc.dram_tensor("w4", [H3, 1], F32, kind="ExternalInput").ap()
    b4 = nc.dram_tensor("b4", [1, 1], F32, kind="ExternalInput").ap()
    y = nc.dram_tensor("y", [BC, 1], F32, kind="ExternalOutput").ap()
    dbg_emb = nc.dram_tensor("dbg_emb", [P, NF * D], F32,
                             kind="ExternalOutput").ap()
    dbg_embD = nc.dram_tensor("dbg_embD", [D, F * P], F32,
                              kind="ExternalOutput").ap()
    dbg_embT = nc.dram_tensor("dbg_embT", [P, 13 * P], F32,
                              kind="ExternalOutput").ap()
    dbg_it = nc.dram_tensor("dbg_it", [P, 7 * P], F32,
                            kind="ExternalOutput").ap()
    dbg_h1 = nc.dram_tensor("dbg_h1", [P, H1], F32,
                            kind="ExternalOutput").ap()

    RELU = mybir.ActivationFunctionType.Relu

    with tile.TileContext(nc) as tc:
        with (
            tc.tile_pool(name="const", bufs=1) as cpool,
            tc.tile_pool(name="wpool", bufs=1) as wpool,
            tc.tile_pool(name="io", bufs=1) as iopool,
            tc.tile_pool(name="work", bufs=2) as work,
            tc.tile_pool(name="pt", bufs=2, space="PSUM") as pt,
            tc.tile_pool(name="pg", bufs=1, space="PSUM") as pg,
            tc.tile_pool(name="pmm", bufs=2, space="PSUM") as pmm,
        ):
            # ---- constants / weights resident in SBUF ----
            ident = cpool.tile([P, P], F32, tag="ident")
            make_identity(nc, ident[:])
            ones = cpool.tile([1, P], F32, tag="ones")
            nc.vector.memset(ones[:], 1.0)

            dw1_sb = wpool.tile([NDENSE, D], F32, tag="dw1")
            nc.sync.dma_start(dw1_sb[:], dw1)
            db1_sb = wpool.tile([D, 1], F32, tag="db1")
            nc.sync.dma_start(db1_sb[:], db1)
            dw2_sb = wpool.tile([D, D], F32, tag="dw2")
            nc.sync.dma_start(dw2_sb[:], dw2)
            db2_sb = wpool.tile([D, 1], F32, tag="db2")
            nc.sync.dma_start(db2_sb[:], db2)

            # w1 packed on host as 21 chunks of [128, 512]
            w1_sb = wpool.tile([P, 21 * H1], F32, tag="w1")
            for c in range(21):
                nc.sync.dma_start(w1_sb[:, c * H1:(c + 1) * H1],
                                  w1[c * P:(c + 1) * P, :])
            b1_sb = wpool.tile([1, H1], F32, tag="b1")
            nc.sync.dma_start(b1_sb[:], b1)
            w2_sb = wpool.tile([P, 4 * H2], F32, tag="w2")
            for c in range(4):
                nc.sync.dma_start(w2_sb[:, c * H2:(c + 1) * H2],
                                  w2[c * P:(c + 1) * P, :])
            b2_sb = wpool.tile([1, H2], F32, tag="b2")
            nc.sync.dma_start(b2_sb[:], b2)
            w3_sb = wpool.tile([P, 2 * H3], F32, tag="w3")
            for c in range(2):
                nc.sync.dma_start(w3_sb[:, c * H3:(c + 1) * H3],
                                  w3[c * P:(c + 1) * P, :])
            b3_sb = wpool.tile([1, H3], F32, tag="b3")
            nc.sync.dma_start(b3_sb[:], b3)
            w4_sb = wpool.tile([H3, 1], F32, tag="w4")
            nc.sync.dma_start(w4_sb[:], w4)
            b4_sb = wpool.tile([1, 1], F32, tag="b4")
            nc.sync.dma_start(b4_sb[:], b4)

            # whole-core dense input + indices resident
            xT_sb = iopool.tile([NDENSE, BC], F32, tag="xT")
            nc.sync.dma_start(xT_sb[:], xT)
            idx_sb = iopool.tile([P, NT * NF], I32, tag="idx")
            for t in range(NT):
                nc.sync.dma_start(idx_sb[:, t * NF:(t + 1) * NF],
                                  idx[t * P:(t + 1) * P, :])
            y_sb = iopool.tile([P, NT], F32, tag="y")

            for t in range(NT):
                # ---- embedding gather: [128, 26, 64] f32 ----
                emb_sb = work.tile([P, NF * D], F32, tag="emb")
                for f in range(NF):
                    nc.gpsimd.indirect_dma_start(
                        out=emb_sb[:, f * D:(f + 1) * D],
                        out_offset=None,
                        in_=tab,
                        in_offset=bass.IndirectOffsetOnAxis(
                            ap=idx_sb[:, t * NF + f:t * NF + f + 1], axis=0),
                    )

                # ---- dense MLP (transposed): embD slot 0 ----
                embD = work.tile([D, F * P], F32, tag="embD")
                ph = pt.tile([P, P], F32, tag="pt", name="pth")
                nc.tensor.matmul(ph[0:D, :], _r(dw1_sb[:]),
                                 _r(xT_sb[:, t * P:(t + 1) * P]),
                                 start=True, stop=True)
                hT = work.tile([D, P], F32, tag="hT")
                nc.scalar.activation(hT[:], ph[0:D, :], RELU, bias=db1_sb[:])
                pd = pt.tile([P, P], F32, tag="pt", name="pth")
                nc.tensor.matmul(pd[0:D, :], _r(dw2_sb[:]), _r(hT[:]),
                                 start=True, stop=True)
                nc.scalar.activation(embD[:, 0:P], pd[0:D, :], RELU,
                                     bias=db2_sb[:])

                # ---- embT: 13 chunk transposes of emb_sb ----
                embT = work.tile([P, 13 * P], F32, tag="embT")
                for c in range(13):
                    ptt = pt.tile([P, P], F32, tag="pt", name="ptt")
                    nc.tensor.transpose(
                        ptt[:], emb_sb[:, c * P:(c + 1) * P], ident[:])
                    _copy(nc, c, embT[:, c * P:(c + 1) * P], ptt[:])

                # ---- embD slots 1..26: per-feature transposes ----
                for f in range(NF):
                    ptf = pt.tile([P, P], F32, tag="pt", name="ptf")
                    nc.tensor.transpose(
                        ptf[0:D, :], emb_sb[:, f * D:(f + 1) * D], ident[:])
                    _copy(nc, f, embD[:, (f + 1) * P:(f + 2) * P], ptf[0:D, :])

                # ---- per-sample grams, 2 rounds of 64 samples ----
                # psum_g [32, 64 slots * 32 f32]; gram b at cols 32b..32b+26
                embD3 = embD[:].rearrange("d (f p) -> d f p", p=P)
                it_t = []
                for j in range(7):
                    itj = work.tile([P if j < 6 else 96, P], F32,
                                    tag=f"it{j}", name=f"it{j}")
                    nc.vector.memset(itj[:], 0.0)
                    it_t.append(itj)
                for r in range(2):
                    pgt = pg.tile([32, 64 * 32], F32, tag="pg")
                    pg3 = pgt[:].rearrange("q (s e) -> q s e", e=32)
                    for s in range(64):
                        b = r * 64 + s
                        op = _r(embD3[:, :, b])
                        nc.tensor.matmul(pg3[0:F, s, 0:F], op, op,
                                         start=True, stop=True)
                    for g in range(F):
                        j, rr = g // 4, g % 4
                        _copy(nc, g,
                              it_t[j][rr * 32:rr * 32 + F,
                                      r * 64:(r + 1) * 64],
                              pg3[0:F, :, g])

                # ---- layer 1: 21 chunk matmuls + bias row ----
                po = pmm.tile([P, H1], F32, tag="po", name="po1")
                for c in range(13):
                    nc.tensor.matmul(
                        po[:], _r(embT[:, c * P:(c + 1) * P]),
                        _r(w1_sb[:, c * H1:(c + 1) * H1]),
                        start=(c == 0), stop=False)
                nc.tensor.matmul(po[:], _r(embD[:, 0:P]),
                                 _r(w1_sb[0:D, 13 * H1:14 * H1]),
                                 start=False, stop=False)
                for j in range(7):
                    kk = P if j < 6 else 96
                    nc.tensor.matmul(
                        po[:], _r(it_t[j][:]),
                        _r(w1_sb[0:kk, (14 + j) * H1:(15 + j) * H1]),
                        start=False, stop=False)
                nc.tensor.matmul(po[:], _r(ones[:]), _r(b1_sb[:]),
                                 start=False, stop=True)
                h1 = work.tile([P, H1], F32, tag="h1")
                nc.scalar.activation(h1[:], po[:], RELU)

                if t == 0:
                    nc.sync.dma_start(dbg_emb, emb_sb[:])
                    nc.sync.dma_start(dbg_embD, embD[:])
                    nc.sync.dma_start(dbg_embT, embT[:])
                    for j in range(6):
                        nc.sync.dma_start(dbg_it[:, j * P:(j + 1) * P].rearrange("p q -> q p"), it_t[j][:])
                    nc.sync.dma_start(dbg_h1, h1[:])

                # ---- layer 2 ----
                h1T = work.tile([P, H1], F32, tag="h1T")
                for c in range(4):
                    ptt = pt.tile([P, P], F32, tag="pt", name="ptt")
                    nc.tensor.transpose(
                        ptt[:], h1[:, c * P:(c + 1) * P], ident[:])
                    _copy(nc, c, h1T[:, c * P:(c + 1) * P], ptt[:])
                po2 = pmm.tile([P, H1], F32, tag="po", name="po2")
                for c in range(4):
                    nc.tensor.matmul(
                        po2[:, 0:H2], _r(h1T[:, c * P:(c + 1) * P]),
                        _r(w2_sb[:, c * H2:(c + 1) * H2]),
                        start=(c == 0), stop=False)
                nc.tensor.matmul(po2[:, 0:H2], _r(ones[:]), _r(b2_sb[:]),
                                 start=False, stop=True)
                h2 = work.tile([P, H2], F32, tag="h2")
                nc.scalar.activation(h2[:], po2[:, 0:H2], RELU)

                # ---- layer 3 ----
                h2T = work.tile([P, H2], F32, tag="h2T")
                for c in range(2):
                    ptt = pt.tile([P, P], F32, tag="pt", name="ptt")
                    nc.tensor.transpose(
                        ptt[:], h2[:, c * P:(c + 1) * P], ident[:])
                    _copy(nc, c, h2T[:, c * P:(c + 1) * P], ptt[:])
                po3 = pmm.tile([P, H1], F32, tag="po", name="po3")
                for c in range(2):
                    nc.tensor.matmul(
                        po3[:, 0:H3], _r(h2T[:, c * P:(c + 1) * P]),
                        _r(w3_sb[:, c * H3:(c + 1) * H3]),
                        start=(c == 0), stop=False)
                nc.tensor.matmul(po3[:, 0:H3], _r(ones[:]), _r(b3_sb[:]),
                                 start=False, stop=True)
                h3 = work.tile([P, H3], F32, tag="h3")
                nc.scalar.activation(h3[:], po3[:, 0:H3], RELU)

                # ---- layer 4 ----
                h3T = work.tile([P, H3], F32, tag="h3T")
                ptt = pt.tile([P, P], F32, tag="pt", name="ptt")
                nc.tensor.transpose(ptt[:], h3[:], ident[:])
                nc.vector.tensor_copy(h3T[:], ptt[:])
                po4 = pmm.tile([P, H1], F32, tag="po", name="po4")
                nc.tensor.matmul(po4[:, 0:1], _r(h3T[:]), _r(w4_sb[:]),
                                 start=True, stop=False)
                nc.tensor.matmul(po4[:, 0:1], _r(ones[:]), _r(b4_sb[:]),
                                 start=False, stop=True)
                nc.vector.tensor_copy(y_sb[:, t:t + 1], po4[:, 0:1])

            for t in range(NT):
                nc.sync.dma_start(y[t * P:(t + 1) * P, :],
                                  y_sb[:, t:t + 1])
    nc.finalize()
    return nc


def _prep_host(inputs):
    """Host-side sharding + weight packing (numpy only)."""
    dense = np.asarray(inputs["dense"], np.float32)
    sparse_idx = np.asarray(inputs["sparse_idx"])
    emb = np.asarray(inputs["emb_tables"], np.float32)
    tab = np.ascontiguousarray(emb.reshape(NF * V, D))

    # flat row index into tab
    fidx = (np.arange(NF, dtype=np.int64)[None, :] * V +
            sparse_idx.astype(np.int64)).astype(np.int32)

    ow1 = np.asarray(inputs["ow1"], np.float32)
    # build packed w1: 21 chunks of 128 rows, zT row map:
    #  chunks 0-12: embT rows r -> ow1 row 64 + r   (sparse emb, natural)
    #  chunk 13 rows 0-63: dense emb -> ow1 rows 0-63
    #  chunks 14-20: inter bands, row g*27+f -> 0.5*ow1[1728+pair(f,g)], diag 0
    w1p = np.zeros((21 * P, H1), np.float32)
    w1p[0:K_EMB] = ow1[D:D + K_EMB]
    w1p[13 * P:13 * P + D] = ow1[0:D]
    iu0, iu1 = np.triu_indices(F, k=1)
    pairpos = {}
    for pp, (a, bb) in enumerate(zip(iu0, iu1)):
        pairpos[(a, bb)] = pp
    for j in range(7):
        nb = 4 if j < 6 else 3
        for rb in range(nb):
            g = 4 * j + rb
            for f in range(F):
                if f == g:
                    continue
                a, bb = (f, g) if f < g else (g, f)
                w1p[(14 + j) * P + rb * 32 + f] = \
                    0.5 * ow1[F * D + pairpos[(a, bb)]]

    def col(v):
        return np.ascontiguousarray(
            np.asarray(v, np.float32).reshape(-1, 1))

    def row(v):
        return np.ascontiguousarray(
            np.asarray(v, np.float32).reshape(1, -1))

    common = {
        "tab": tab,
        "dw1": np.ascontiguousarray(np.asarray(inputs["dw1"], np.float32)),
        "db1": col(inputs["db1"]),
        "dw2": np.ascontiguousarray(np.asarray(inputs["dw2"], np.float32)),
        "db2": col(inputs["db2"]),
        "w1": w1p,
        "b1": row(inputs["ob1"]),
        "w2": np.ascontiguousarray(np.asarray(inputs["ow2"], np.float32)),
        "b2": row(inputs["ob2"]),
        "w3": np.ascontiguousarray(np.asarray(inputs["ow3"], np.float32)),
        "b3": row(inputs["ob3"]),
        "w4": np.ascontiguousarray(np.asarray(inputs["ow4"], np.float32)),
        "b4": row(inputs["ob4"]),
    }
    in_maps = []
    for c in range(NCORES):
        sl = slice(c * BC, (c + 1) * BC)
        m = dict(common)
        m["xT"] = np.ascontiguousarray(dense[sl].T)
        m["idx"] = np.ascontiguousarray(fidx[sl])
        in_maps.append(m)
    return in_maps


def kernel(**inputs):
    if "nc" not in _cache:
        _cache["nc"] = build_nc()
    nc = _cache["nc"]
    in_maps = _prep_host(inputs)
    res = run_bass_kernel_spmd(nc, in_maps, core_ids=list(range(NCORES)))
    outs = res.results
    return np.concatenate([outs[c]["y"] for c in range(NCORES)], axis=0)


if __name__ == "__main__":
    rng = np.random.default_rng(0)
    fake = {
        "dense": rng.standard_normal((B, NDENSE), dtype=np.float32),
        "sparse_idx": rng.integers(0, V, (B, NF)).astype(np.int32),
        "emb_tables": rng.standard_normal((NF, V, D), dtype=np.float32) * 0.05,
        "dw1": rng.standard_normal((NDENSE, D), dtype=np.float32) * 0.05,
        "db1": np.zeros(D, np.float32),
        "dw2": rng.standard_normal((D, D), dtype=np.float32) * 0.05,
        "db2": np.zeros(D, np.float32),
        "ow1": rng.standard_normal((F * D + NF * F // 2, H1),
                                   dtype=np.float32) * 0.05,
        "ob1": np.zeros(H1, np.float32),
        "ow2": rng.standard_normal((H1, H2), dtype=np.float32) * 0.05,
        "ob2": np.zeros(H2, np.float32),
        "ow3": rng.standard_normal((H2, H3), dtype=np.float32) * 0.05,
        "ob3": np.zeros(H3, np.float32),
        "ow4": rng.standard_normal((H3, 1), dtype=np.float32) * 0.05,
        "ob4": np.zeros(1, np.float32),
    }
    out = kernel(**fake)
    print(out.shape, out.dtype, out[:4, 0])

